# revision 25
# baseline (speedup 1.0000x reference)
"""Trainium2 Bass kernel for nn_BaselineDistiller: grouped-expert MLP + MSE loss.

reference:
    h    = einsum('bne,neh->bnh', features, W1) + b1
    g    = gelu(h)                      # exact (erf) gelu
    pred = einsum('bnh,nhe->bne', g, W2) + b2
    out  = mean((pred - target)^2)

Strategy (8 NeuronCores, data-parallel over batch; ~151us on HW, vs 160us
for the bf16 predecessor):
  * Host: shard batch 8-ways; pre-transpose activations to expert-major
    [NE, E, 2, B_shard] fp8-e4m3 (feat and b2-folded target interleaved per
    partition row so one DMA per expert carries both, all on the SP hardware
    DMA queue). W1 stays bf16 (mm1 runs mixed bf16 lhsT x fp8 rhs -- verified
    exact on HW), W2 is fp8 packed [128, 2, E] for DoubleRow.
  * Device (per core, per expert, per pair of 512-col batch tiles):
      mm1 (2 H-chunks x 2 tiles)           -> h.T in PSUM (f32)
      ACT gelu(+b1 per-partition bias), one op per chunk over the pair
        (free dim 1024), output fp8 hact [128, tile, chunk, 512]
      mm2 accumulation group per tile: ONE DoubleRow fp8 matmul (K=2x128
        contracts both H-chunks in one pass, 2x PE throughput) + a
        (-I bf16) @ targ.T(fp8) matmul so PSUM holds pred.T - target.T
        (the subtract costs PE, not DVE)
      DVE bn_stats per diff tile -> per-partition {count, mean, M2} pairs
    The loop is software-pipelined (next pair's mm1 emitted before the
    current pair's mm2) so the in-order PE never idles waiting on gelu.
    A warmup block (DVE memset -> 8 dummy matmuls -> 1 dummy gelu) runs
    during the NRT preamble/first DMAs: it ramps the PE p-state and pulls
    the ~1.5us gelu ACT_TABLE_LOAD off the critical path, leaving the
    128-op ScalarE gelu stream (~132us span) fully gapless -- that stream
    is the saturated bottleneck engine (1 elem/lane/cycle at 1.2 GHz is a
    hard floor; FD=1024 per op is the largest PSUM double-buffering
    allows: 2 chunk regions + 2 pred banks = all 8 PSUM banks). PE
    (mm1 256 + mm2-DR 128 + negI 128 ops, ~128us) and DMA (~21 MB/core,
    one ~150 GB/s queue) run just under it. First expert splits feat/head/
    targ across the SP and ACT hardware queue families so mm1's inputs
    don't queue behind the full 512KB expert transfer. The remaining
    ~25us is NRT preamble (~7.6us), DMA/compute ramp, and the fixed NEFF
    exit barrier (~10us: the postamble clears ~250 semaphores, ~57
    EVENT_SEMAPHOREs per engine, regardless of kernel structure).
  * Host: sum of squares = sum over tiles of M2s + n*mean^2, reduced in f64,
    divided by the element count.

  Measured on this toolchain (bench_dr*.py):
    - matmul FD=512 costs ~218-262ns regardless of dtype/perf-mode (issue/
      ldweights-bound, 1 col/cycle); DoubleRow wins only by op-count (K=256
      per pass). Matmul out crossing a PSUM bank (FD>512 f32) is an ISA
      error. bn_stats FD is capped at 512 (hw limit).
    - ACTIVATE FD=1024 ~1110ns, FD=2048 ~1860ns (strided 2-region APs cost
      the same as contiguous) -- but 2x8KB gelu regions + pred banks exceed
      the 16KB PSUM, so FD=2048 schemes serialize the pipeline and lose.
    - gelu -> fp8 output is exact; mixed bf16 x fp8 matmul is exact.
    - walrus --enable-ldw-opt errors out on bass-emitted matmuls; shrinking
      DMAQueue num_queues stalls the rings (361us); DVE cannot issue DMAs.
"""

import contextlib
import ctypes
import json
import sys
import types

import ml_dtypes
import numpy as np

import concourse.bass as bass
import concourse.mybir as mybir
import concourse.tile as tile
from concourse import bass_utils
from concourse.bass import ts
from concourse.bass_utils import run_bass_kernel_spmd

B, NE, E, H = 16384, 32, 128, 256
C = 8              # cores
BS = B // C        # batch rows per core
BT = 512           # batch columns per matmul tile
NT = BS // BT
BF16 = mybir.dt.bfloat16
F32 = mybir.dt.float32
FP8 = mybir.dt.float8e4
DR = mybir.MatmulPerfMode.DoubleRow

# ---------------------------------------------------------------------------
# Environment shims (idempotent):
#  1. antenv.axon_hooks — the image's antenv lacks it; provide the NTFF
#     profile hook via ctypes so trace=True works when a caller requests it.
#  2. upload_artifacts — no bucket access in this container; keep local.
#  3. This walrus build rejects instructions with >1 sync-wait; split the
#     extra waits onto NoOps at BIR-serialization time.
# ---------------------------------------------------------------------------
_AXON_SO = "/opt/axon/libaxon_pjrt.so"


def _make_ntff_hook(so_path):
    try:
        lib = ctypes.CDLL(so_path)
    except OSError:
        return None
    if not hasattr(lib, "axon_start_nrt_profile"):
        return None
    lib.axon_start_nrt_profile.argtypes = [ctypes.POINTER(ctypes.c_int64), ctypes.c_size_t]
    lib.axon_start_nrt_profile.restype = ctypes.c_int64
    lib.axon_stop_nrt_profile.argtypes = [ctypes.c_char_p]
    lib.axon_stop_nrt_profile.restype = ctypes.c_int64

    @contextlib.contextmanager
    def _hook(output_dir, device_ids):
        import jax

        jax.devices()
        if device_ids:
            ids = (ctypes.c_int64 * len(device_ids))(*device_ids)
            rc = lib.axon_start_nrt_profile(ids, len(device_ids))
        else:
            rc = lib.axon_start_nrt_profile(None, 0)
        if rc != 0:
            raise RuntimeError(f"axon_start_nrt_profile rc={rc}")
        try:
            yield
        finally:
            n = lib.axon_stop_nrt_profile(str(output_dir).encode())
            print(f"profile: {n} file(s) written to {output_dir}", file=sys.stderr)

    return _hook


if "antenv.axon_hooks" not in sys.modules:
    _mod = types.ModuleType("antenv.axon_hooks")
    _the_hook = _make_ntff_hook(_AXON_SO)
    _mod.get_axon_ntff_profile_hook = lambda: _the_hook
    sys.modules["antenv.axon_hooks"] = _mod

bass_utils.upload_artifacts = lambda tmpdir: str(tmpdir)

_MAXW = 1
if not getattr(bass.Bass, "_wait_split_installed", False):
    _orig_to_json_bytes = bass.Bass.to_json_bytes

    def _split_sync_waits(self, *a, **kw):
        bir = json.loads(_orig_to_json_bytes(self, *a, **kw))
        for fn in bir.get("functions", []):
            for blk in fn.get("blocks", []):
                new_insts = []
                for inst in blk.get("instructions", []):
                    si = inst.get("sync_info") or {}
                    waits = si.get("on_wait") or []
                    if len(waits) > _MAXW:
                        extra, keep = waits[:-_MAXW], waits[-_MAXW:]
                        for k in range(0, len(extra), _MAXW):
                            new_insts.append({
                                "debug": inst.get("debug", 0),
                                "engine": inst["engine"],
                                "ins": [], "outs": [],
                                "name": f"{inst['name']}_wsplit{k}",
                                "opcode": "NoOp",
                                "sync_info": {"on_update": [],
                                              "on_wait": extra[k:k + _MAXW]},
                            })
                        si["on_wait"] = keep
                    new_insts.append(inst)
                blk["instructions"] = new_insts
        return json.dumps(bir).encode()

    bass.Bass.to_json_bytes = _split_sync_waits
    bass.Bass._wait_split_installed = True


# ---------------------------------------------------------------------------
# Device kernel
# ---------------------------------------------------------------------------
NTILES = NE * NT          # batch tiles, per core
STATS_DIM = 6


def _build_nc():
    nc = bass.Bass("TRN2", target_bir_lowering=False, debug=False)
    ftd = nc.declare_dram_parameter("ft", [NE, E, 2, BS], FP8, isOutput=False)
    w1d = nc.declare_dram_parameter("w1", [E, NE, H], BF16, isOutput=False)
    w2d = nc.declare_dram_parameter("w2", [128, NE, 2, E], FP8, isOutput=False)
    headd = nc.declare_dram_parameter("head", [128, 512], BF16, isOutput=False)
    statsd = nc.declare_dram_parameter("stats", [128, NTILES, STATS_DIM], F32,
                                       isOutput=True)

    with tile.TileContext(nc) as tc, contextlib.ExitStack() as ctx:
        wpool = ctx.enter_context(tc.tile_pool(name="weights", bufs=1))
        iopool = ctx.enter_context(tc.tile_pool(name="io", bufs=4))
        hpool = ctx.enter_context(tc.tile_pool(name="h", bufs=4))
        stpool = ctx.enter_context(tc.tile_pool(name="stats", bufs=1))
        ph0p = ctx.enter_context(tc.tile_pool(name="ph0", bufs=1, space="PSUM"))
        ph1p = ctx.enter_context(tc.tile_pool(name="ph1", bufs=1, space="PSUM"))
        ppp = ctx.enter_context(tc.tile_pool(name="pp", bufs=2, space="PSUM"))

        # Packed head tile = [expert-0 W1 | -I | b1-as-bits]: one DMA
        # unblocks the first matmuls and gelu bias. Expert-0 W2 rides the
        # first w2 group DMA (needed ~3us later, arrives ~2us in).
        head_sb = wpool.tile([128, 512], BF16)
        negi_sb = head_sb[:, 256:384]
        b1f = head_sb[:, 384:512].bitcast(F32)       # [128, 64] = b1[2, NE]
        GE = 4                      # experts per weight-DMA group
        NG = NE // GE
        GS = 4                      # experts per stats-out group
        w1g, w2g = [], []
        for g in range(NG):
            w1g.append(wpool.tile([E, GE, H], BF16, name=f"w1g{g}"))
            w2g.append(wpool.tile([128, GE, 2, E], FP8, name=f"w2g{g}"))

        stats_sb = stpool.tile([128, NTILES, STATS_DIM], F32)

        # Warmup while the first DMAs are in flight: 8 FD=512 matmuls hold
        # the PE busy ~3.5us so it reaches the full 2.4GHz p-state before
        # the first real mm1 (a shorter warmup leaves expert 0 at the mid
        # p-state and costs more than it saves), and a dummy gelu pulls the
        # ~1.5us ACT table load off the critical path. No data deps.
        warm = wpool.tile([128, 512], BF16, name="warm")
        nc.vector.memset(warm[:], 0.0)
        warm_pp = ppp.tile([128, BT], F32, name="pp0")
        for _ in range(8):
            nc.tensor.matmul(warm_pp[:], lhsT=warm[:, 0:128], rhs=warm[:],
                             start=True, stop=True)
        warm_out = wpool.tile([128, 512], BF16, name="warmo")
        nc.scalar.activation(warm_out[:], warm_pp[:],
                             mybir.ActivationFunctionType.Gelu, scale=1.0)

        # Software-pipelined over pairs of 512-col batch tiles: emit the NEXT
        # pair's mm1 before the CURRENT pair's mm2 so the PE (in-order) can
        # fill its gelu-wait with independent work.
        pending = None   # (hact, targ, n, t0, t1) awaiting mm2+bn_stats

        def flush(pending):
            hact, targ, n, t0, t1 = pending
            w2n = w2g[n // GE][:, n % GE]  # [128, 2, E]
            pp0 = ppp.tile([128, BT], F32, name="pp0")
            pp1 = ppp.tile([128, BT], F32, name="pp1")
            for pp_i, i in ((pp0, 0), (pp1, 1)):
                nc.tensor.matmul(pp_i[:], lhsT=w2n, rhs=hact[:, :, i, :],
                                 start=True, stop=False,
                                 perf_mode=DR, skip_group_check=True)
            for pp_i, t in ((pp0, t0), (pp1, t1)):
                nc.tensor.matmul(pp_i[:], lhsT=negi_sb,
                                 rhs=targ[:, ts(t, BT)],
                                 start=False, stop=True,
                                 skip_group_check=True)
            for pp_i, t in ((pp0, t0), (pp1, t1)):
                nc.vector.bn_stats(out=stats_sb[:, n * NT + t, :], in_=pp_i[:])

        for n in range(NE):
            if n == 0:
                # First expert: split feat/targ/head across the SP and ACT
                # hardware queue families so mm1's inputs (head + feat) don't
                # queue behind the full 512KB expert transfer.
                fa0 = iopool.tile([E, BS], FP8, name="fa0")
                ta0 = iopool.tile([E, BS], FP8, name="ta0")
                nc.scalar.dma_start(out=head_sb[:], in_=headd[:])
                nc.sync.dma_start(out=fa0[:], in_=ftd[0, :, 0, :])
                nc.scalar.dma_start(out=ta0[:], in_=ftd[0, :, 1, :])
                nc.sync.dma_start(out=w1g[0][:], in_=w1d[:, 0:GE, :])
                nc.sync.dma_start(out=w2g[0][:], in_=w2d[:, 0:GE, :, :])
                feat = fa0[:, :]
                featp = None
                targ = ta0[:, :]
            else:
                ft_sb = iopool.tile([E, 2, BS], FP8, tag="ft")
                nc.sync.dma_start(out=ft_sb[:], in_=ftd[n])
                feat = ft_sb[:, 0, :]
                featp = None
                targ = ft_sb[:, 1, :]
            if n % GE == 1:
                g = n // GE + 1
                if g < NG:
                    nc.sync.dma_start(out=w1g[g][:], in_=w1d[:, ts(g, GE), :])
            if n % GE == 2:
                g = n // GE + 1
                if g < NG:
                    nc.sync.dma_start(out=w2g[g][:], in_=w2d[:, ts(g, GE), :, :])
            if n % GS == 2 and n > GS:
                # experts <= n-2 have flushed; ship the previous group's stats
                gd = n // GS - 1
                nc.sync.dma_start(out=statsd[:, ts(gd, GS * NT), :],
                                  in_=stats_sb[:, ts(gd, GS * NT), :])
            if n == NE - 1:
                # experts 24..27 have flushed by now
                gd = NE // GS - 2
                nc.sync.dma_start(out=statsd[:, ts(gd, GS * NT), :],
                                  in_=stats_sb[:, ts(gd, GS * NT), :])
            for tp in range(NT // 2):
                t0, t1 = 2 * tp, 2 * tp + 1
                # mm1: h.T chunks; one weight load serves both tiles of a pair
                ph = [None, None]
                for c, pool_c in ((0, ph0p), (1, ph1p)):
                    ph[c] = pool_c.tile([128, 2, BT], F32, name=f"ph{c}")
                    for i, t in enumerate((t0, t1)):
                        nc.tensor.matmul(
                            ph[c][:, i, :],
                            lhsT=(head_sb[:, ts(c, 128)] if n == 0 else w1g[n // GE][:, n % GE, ts(c, 128)]),
                            rhs=(featp[t // 2][:, ts(t % 2, BT)]
                                 if featp is not None else feat[:, ts(t, BT)]),
                            start=True, stop=True,
                        )
                if pending is not None:
                    flush(pending)
                # gelu(+b1): one ACT op per chunk over both tiles (FD=1024),
                # fp8 output laid out [chunk, tile, BT]: the ACT write is
                # contiguous; the DoubleRow rhs [128, 2(chunk), BT] slice is
                # the strided side (PE access patterns are free).
                hact = hpool.tile([128, 2, 2, BT], FP8)   # [c, tile, BT]
                for c in range(2):
                    nc.scalar.activation(
                        hact[:, c], ph[c][:, :, :],
                        mybir.ActivationFunctionType.Gelu,
                        bias=b1f[:, c * NE + n:c * NE + n + 1], scale=1.0,
                    )
                pending = (hact, targ, n, t0, t1)
        # ship experts 28..30 as soon as expert 30 flushes (during expert 31's
        # mm2), leaving only expert 31's 4 tiles for the final transfer
        nc.sync.dma_start(out=statsd[:, (NE - 4) * NT:(NE - 1) * NT, :],
                          in_=stats_sb[:, (NE - 4) * NT:(NE - 1) * NT, :])
        flush(pending)
        nc.sync.dma_start(out=statsd[:, (NE - 1) * NT:, :],
                          in_=stats_sb[:, (NE - 1) * NT:, :])
    return nc


LAST_RESULTS = None


def kernel(features, target_features, W1, b1, W2, b2):
    global LAST_RESULTS
    bf = ml_dtypes.bfloat16
    f8 = ml_dtypes.float8_e4m3
    features = np.asarray(features)
    target_features = np.asarray(target_features)
    W1 = np.asarray(W1)
    b1 = np.asarray(b1)
    W2 = np.asarray(W2)
    b2 = np.asarray(b2)

    # [C, NE, E, 2, BS] fp8: feat/targ interleaved per partition row
    feat4 = features.reshape(C, BS, NE, E).transpose(0, 2, 3, 1)
    targ4 = (target_features - b2[None]).reshape(C, BS, NE, E).transpose(0, 2, 3, 1)
    ft = np.stack([feat4, targ4], axis=3).astype(f8)   # [C, NE, E, 2, BS]
    w1h = W1.transpose(1, 0, 2).astype(bf)                          # [E, NE, H]
    w2h = W2.reshape(NE, 2, 128, E).transpose(2, 0, 1, 3).astype(f8)  # [128, NE, 2, E]
    b1h = np.ascontiguousarray(b1.reshape(NE, 2, 128).transpose(2, 1, 0).astype(np.float32))

    negi = (-np.eye(128)).astype(bf)
    head = np.ascontiguousarray(np.concatenate(
        [w1h[:, 0, :].view(np.uint16), negi.view(np.uint16),
         b1h.reshape(128, 64).view(np.uint16)],
        axis=1)).view(bf)

    nc = _build_nc()
    in_maps = [
        {"ft": np.ascontiguousarray(ft[c]),
         "w1": w1h, "w2": w2h, "head": head}
        for c in range(C)
    ]
    res = run_bass_kernel_spmd(nc, in_maps, list(range(C)))
    LAST_RESULTS = res
    # stats[p, tile] = [n0, mean0, M2_0, n1, mean1, M2_1] of the diff rows
    # (bn_stats splits the 512 free elems into two 256-halves);
    # sum of squares = M2_0 + n0*mean0^2 + M2_1 + n1*mean1^2.
    total = 0.0
    for r in res.results:
        st = r["stats"].astype(np.float64)
        total += (st[..., 2] + st[..., 0] * st[..., 1] ** 2
                  + st[..., 5] + st[..., 3] * st[..., 4] ** 2).sum()
    return np.array(total / (B * NE * E), dtype=np.float32)


# revision 26
# speedup vs baseline: 1.0012x; 1.0012x over previous
"""Trainium2 Bass kernel for nn_BaselineDistiller: grouped-expert MLP + MSE loss.

reference:
    h    = einsum('bne,neh->bnh', features, W1) + b1
    g    = gelu(h)                      # exact (erf) gelu
    pred = einsum('bnh,nhe->bne', g, W2) + b2
    out  = mean((pred - target)^2)

Strategy (8 NeuronCores, data-parallel over batch; ~151us on HW, vs 160us
for the bf16 predecessor):
  * Host: shard batch 8-ways; pre-transpose activations to expert-major
    [NE, E, 2, B_shard] fp8-e4m3 (feat and b2-folded target interleaved per
    partition row so one DMA per expert carries both, all on the SP hardware
    DMA queue). W1 stays bf16 (mm1 runs mixed bf16 lhsT x fp8 rhs -- verified
    exact on HW), W2 is fp8 packed [128, 2, E] for DoubleRow.
  * Device (per core, per expert, per pair of 512-col batch tiles):
      mm1 (2 H-chunks x 2 tiles)           -> h.T in PSUM (f32)
      ACT gelu(+b1 per-partition bias), one op per chunk over the pair
        (free dim 1024), output fp8 hact [128, tile, chunk, 512]
      mm2 accumulation group per tile: ONE DoubleRow fp8 matmul (K=2x128
        contracts both H-chunks in one pass, 2x PE throughput) + a
        (-I bf16) @ targ.T(fp8) matmul so PSUM holds pred.T - target.T
        (the subtract costs PE, not DVE)
      DVE bn_stats per diff tile -> per-partition {count, mean, M2} pairs
    The loop is software-pipelined (next pair's mm1 emitted before the
    current pair's mm2) so the in-order PE never idles waiting on gelu.
    A warmup block (DVE memset -> 8 dummy matmuls -> 1 dummy gelu) runs
    during the NRT preamble/first DMAs: it ramps the PE p-state and pulls
    the ~1.5us gelu ACT_TABLE_LOAD off the critical path, leaving the
    128-op ScalarE gelu stream (~132us span) fully gapless -- that stream
    is the saturated bottleneck engine (1 elem/lane/cycle at 1.2 GHz is a
    hard floor; FD=1024 per op is the largest PSUM double-buffering
    allows: 2 chunk regions + 2 pred banks = all 8 PSUM banks). PE
    (mm1 256 + mm2-DR 128 + negI 128 ops, ~128us) and DMA (~21 MB/core,
    one ~150 GB/s queue) run just under it. First expert splits feat/head/
    targ across the SP and ACT hardware queue families so mm1's inputs
    don't queue behind the full 512KB expert transfer. The remaining
    ~25us is NRT preamble (~7.6us), DMA/compute ramp, and the fixed NEFF
    exit barrier (~10us: the postamble clears ~250 semaphores, ~57
    EVENT_SEMAPHOREs per engine, regardless of kernel structure).
  * Host: sum of squares = sum over tiles of M2s + n*mean^2, reduced in f64,
    divided by the element count.

  Measured on this toolchain (bench_dr*.py):
    - matmul FD=512 costs ~218-262ns regardless of dtype/perf-mode (issue/
      ldweights-bound, 1 col/cycle); DoubleRow wins only by op-count (K=256
      per pass). Matmul out crossing a PSUM bank (FD>512 f32) is an ISA
      error. bn_stats FD is capped at 512 (hw limit).
    - ACTIVATE FD=1024 ~1110ns, FD=2048 ~1860ns (strided 2-region APs cost
      the same as contiguous) -- but 2x8KB gelu regions + pred banks exceed
      the 16KB PSUM, so FD=2048 schemes serialize the pipeline and lose.
    - gelu -> fp8 output is exact; mixed bf16 x fp8 matmul is exact.
    - walrus --enable-ldw-opt errors out on bass-emitted matmuls; shrinking
      DMAQueue num_queues stalls the rings (361us); DVE cannot issue DMAs.
"""

import contextlib
import ctypes
import json
import sys
import types

import ml_dtypes
import numpy as np

import concourse.bass as bass
import concourse.mybir as mybir
import concourse.tile as tile
from concourse import bass_utils
from concourse.bass import ts
from concourse.bass_utils import run_bass_kernel_spmd

B, NE, E, H = 16384, 32, 128, 256
C = 8              # cores
BS = B // C        # batch rows per core
BT = 512           # batch columns per matmul tile
NT = BS // BT
BF16 = mybir.dt.bfloat16
F32 = mybir.dt.float32
FP8 = mybir.dt.float8e4
DR = mybir.MatmulPerfMode.DoubleRow

# ---------------------------------------------------------------------------
# Environment shims (idempotent):
#  1. antenv.axon_hooks — the image's antenv lacks it; provide the NTFF
#     profile hook via ctypes so trace=True works when a caller requests it.
#  2. upload_artifacts — no bucket access in this container; keep local.
#  3. This walrus build rejects instructions with >1 sync-wait; split the
#     extra waits onto NoOps at BIR-serialization time.
# ---------------------------------------------------------------------------
_AXON_SO = "/opt/axon/libaxon_pjrt.so"


def _make_ntff_hook(so_path):
    try:
        lib = ctypes.CDLL(so_path)
    except OSError:
        return None
    if not hasattr(lib, "axon_start_nrt_profile"):
        return None
    lib.axon_start_nrt_profile.argtypes = [ctypes.POINTER(ctypes.c_int64), ctypes.c_size_t]
    lib.axon_start_nrt_profile.restype = ctypes.c_int64
    lib.axon_stop_nrt_profile.argtypes = [ctypes.c_char_p]
    lib.axon_stop_nrt_profile.restype = ctypes.c_int64

    @contextlib.contextmanager
    def _hook(output_dir, device_ids):
        import jax

        jax.devices()
        if device_ids:
            ids = (ctypes.c_int64 * len(device_ids))(*device_ids)
            rc = lib.axon_start_nrt_profile(ids, len(device_ids))
        else:
            rc = lib.axon_start_nrt_profile(None, 0)
        if rc != 0:
            raise RuntimeError(f"axon_start_nrt_profile rc={rc}")
        try:
            yield
        finally:
            n = lib.axon_stop_nrt_profile(str(output_dir).encode())
            print(f"profile: {n} file(s) written to {output_dir}", file=sys.stderr)

    return _hook


if "antenv.axon_hooks" not in sys.modules:
    _mod = types.ModuleType("antenv.axon_hooks")
    _the_hook = _make_ntff_hook(_AXON_SO)
    _mod.get_axon_ntff_profile_hook = lambda: _the_hook
    sys.modules["antenv.axon_hooks"] = _mod

bass_utils.upload_artifacts = lambda tmpdir: str(tmpdir)

_MAXW = 1
if not getattr(bass.Bass, "_wait_split_installed", False):
    _orig_to_json_bytes = bass.Bass.to_json_bytes

    def _split_sync_waits(self, *a, **kw):
        bir = json.loads(_orig_to_json_bytes(self, *a, **kw))
        for fn in bir.get("functions", []):
            for blk in fn.get("blocks", []):
                new_insts = []
                for inst in blk.get("instructions", []):
                    si = inst.get("sync_info") or {}
                    waits = si.get("on_wait") or []
                    if len(waits) > _MAXW:
                        extra, keep = waits[:-_MAXW], waits[-_MAXW:]
                        for k in range(0, len(extra), _MAXW):
                            new_insts.append({
                                "debug": inst.get("debug", 0),
                                "engine": inst["engine"],
                                "ins": [], "outs": [],
                                "name": f"{inst['name']}_wsplit{k}",
                                "opcode": "NoOp",
                                "sync_info": {"on_update": [],
                                              "on_wait": extra[k:k + _MAXW]},
                            })
                        si["on_wait"] = keep
                    new_insts.append(inst)
                blk["instructions"] = new_insts
        return json.dumps(bir).encode()

    bass.Bass.to_json_bytes = _split_sync_waits
    bass.Bass._wait_split_installed = True


# ---------------------------------------------------------------------------
# Device kernel
# ---------------------------------------------------------------------------
NTILES = NE * NT          # batch tiles, per core
STATS_DIM = 6


def _build_nc():
    nc = bass.Bass("TRN2", target_bir_lowering=False, debug=False)
    ftd = nc.declare_dram_parameter("ft", [NE, E, 2, BS], FP8, isOutput=False)
    w1d = nc.declare_dram_parameter("w1", [E, NE, H], BF16, isOutput=False)
    w2d = nc.declare_dram_parameter("w2", [128, NE, 2, E], FP8, isOutput=False)
    headd = nc.declare_dram_parameter("head", [128, 512], BF16, isOutput=False)
    statsd = nc.declare_dram_parameter("stats", [128, NTILES, STATS_DIM], F32,
                                       isOutput=True)

    with tile.TileContext(nc) as tc, contextlib.ExitStack() as ctx:
        wpool = ctx.enter_context(tc.tile_pool(name="weights", bufs=1))
        iopool = ctx.enter_context(tc.tile_pool(name="io", bufs=5))
        hpool = ctx.enter_context(tc.tile_pool(name="h", bufs=6))
        stpool = ctx.enter_context(tc.tile_pool(name="stats", bufs=1))
        ph0p = ctx.enter_context(tc.tile_pool(name="ph0", bufs=1, space="PSUM"))
        ph1p = ctx.enter_context(tc.tile_pool(name="ph1", bufs=1, space="PSUM"))
        ppp = ctx.enter_context(tc.tile_pool(name="pp", bufs=2, space="PSUM"))

        # Packed head tile = [expert-0 W1 | -I | b1-as-bits]: one DMA
        # unblocks the first matmuls and gelu bias. Expert-0 W2 rides the
        # first w2 group DMA (needed ~3us later, arrives ~2us in).
        head_sb = wpool.tile([128, 512], BF16)
        negi_sb = head_sb[:, 256:384]
        b1f = head_sb[:, 384:512].bitcast(F32)       # [128, 64] = b1[2, NE]
        GE = 4                      # experts per weight-DMA group
        NG = NE // GE
        GS = 4                      # experts per stats-out group
        w1g, w2g = [], []
        for g in range(NG):
            w1g.append(wpool.tile([E, GE, H], BF16, name=f"w1g{g}"))
            w2g.append(wpool.tile([128, GE, 2, E], FP8, name=f"w2g{g}"))

        stats_sb = stpool.tile([128, NTILES, STATS_DIM], F32)

        # Warmup while the first DMAs are in flight: 8 FD=512 matmuls hold
        # the PE busy ~3.5us so it reaches the full 2.4GHz p-state before
        # the first real mm1 (a shorter warmup leaves expert 0 at the mid
        # p-state and costs more than it saves), and a dummy gelu pulls the
        # ~1.5us ACT table load off the critical path. No data deps.
        warm = wpool.tile([128, 512], BF16, name="warm")
        nc.vector.memset(warm[:], 0.0)
        warm_pp = ppp.tile([128, BT], F32, name="pp0")
        for _ in range(8):
            nc.tensor.matmul(warm_pp[:], lhsT=warm[:, 0:128], rhs=warm[:],
                             start=True, stop=True)
        warm_out = wpool.tile([128, 512], BF16, name="warmo")
        nc.scalar.activation(warm_out[:], warm_pp[:],
                             mybir.ActivationFunctionType.Gelu, scale=1.0)

        # Software-pipelined over pairs of 512-col batch tiles: emit the NEXT
        # pair's mm1 before the CURRENT pair's mm2 so the PE (in-order) can
        # fill its gelu-wait with independent work.
        pending = None   # (hact, targ, n, t0, t1) awaiting mm2+bn_stats

        def flush(pending):
            hact, targ, n, t0, t1 = pending
            w2n = w2g[n // GE][:, n % GE]  # [128, 2, E]
            pp0 = ppp.tile([128, BT], F32, name="pp0")
            pp1 = ppp.tile([128, BT], F32, name="pp1")
            for pp_i, i in ((pp0, 0), (pp1, 1)):
                nc.tensor.matmul(pp_i[:], lhsT=w2n, rhs=hact[:, i],
                                 start=True, stop=False,
                                 perf_mode=DR, skip_group_check=True)
            for pp_i, t in ((pp0, t0), (pp1, t1)):
                nc.tensor.matmul(pp_i[:], lhsT=negi_sb,
                                 rhs=targ[:, ts(t, BT)],
                                 start=False, stop=True,
                                 skip_group_check=True)
            for pp_i, t in ((pp0, t0), (pp1, t1)):
                nc.vector.bn_stats(out=stats_sb[:, n * NT + t, :], in_=pp_i[:])

        for n in range(NE):
            if n == 0:
                # First expert: split feat/targ/head across the SP and ACT
                # hardware queue families so mm1's inputs (head + feat) don't
                # queue behind the full 512KB expert transfer.
                fa0 = iopool.tile([E, BS], FP8, name="fa0")
                ta0 = iopool.tile([E, BS], FP8, name="ta0")
                nc.scalar.dma_start(out=head_sb[:], in_=headd[:])
                nc.sync.dma_start(out=fa0[:], in_=ftd[0, :, 0, :])
                nc.scalar.dma_start(out=ta0[:], in_=ftd[0, :, 1, :])
                nc.sync.dma_start(out=w1g[0][:], in_=w1d[:, 0:GE, :])
                nc.sync.dma_start(out=w2g[0][:], in_=w2d[:, 0:GE, :, :])
                feat = fa0[:, :]
                featp = None
                targ = ta0[:, :]
            else:
                ft_sb = iopool.tile([E, 2, BS], FP8, tag="ft")
                nc.sync.dma_start(out=ft_sb[:], in_=ftd[n])
                feat = ft_sb[:, 0, :]
                featp = None
                targ = ft_sb[:, 1, :]
            if n % GE == 1:
                g = n // GE + 1
                if g < NG:
                    nc.sync.dma_start(out=w1g[g][:], in_=w1d[:, ts(g, GE), :])
            if n % GE == 2:
                g = n // GE + 1
                if g < NG:
                    nc.sync.dma_start(out=w2g[g][:], in_=w2d[:, ts(g, GE), :, :])
            if n % GS == 2 and n > GS:
                # experts <= n-2 have flushed; ship the previous group's stats
                gd = n // GS - 1
                nc.sync.dma_start(out=statsd[:, ts(gd, GS * NT), :],
                                  in_=stats_sb[:, ts(gd, GS * NT), :])
            if n == NE - 1:
                # experts 24..27 have flushed by now
                gd = NE // GS - 2
                nc.sync.dma_start(out=statsd[:, ts(gd, GS * NT), :],
                                  in_=stats_sb[:, ts(gd, GS * NT), :])
            for tp in range(NT // 2):
                t0, t1 = 2 * tp, 2 * tp + 1
                # mm1: h.T chunks; one weight load serves both tiles of a pair
                ph = [None, None]
                for c, pool_c in ((0, ph0p), (1, ph1p)):
                    ph[c] = pool_c.tile([128, 2, BT], F32, name=f"ph{c}")
                    for i, t in enumerate((t0, t1)):
                        nc.tensor.matmul(
                            ph[c][:, i, :],
                            lhsT=(head_sb[:, ts(c, 128)] if n == 0 else w1g[n // GE][:, n % GE, ts(c, 128)]),
                            rhs=(featp[t // 2][:, ts(t % 2, BT)]
                                 if featp is not None else feat[:, ts(t, BT)]),
                            start=True, stop=True,
                        )
                if pending is not None:
                    flush(pending)
                # gelu(+b1): one ACT op per chunk over both tiles (FD=1024),
                # fp8 output laid out [tile, chunk, BT] so each tile's rhs for
                # the DoubleRow mm2 is a [128, 2, BT] slice.
                hact = hpool.tile([128, 2, 2, BT], FP8)   # [tile, c, BT]
                for c in range(2):
                    nc.scalar.activation(
                        hact[:, :, c, :], ph[c][:, :, :],
                        mybir.ActivationFunctionType.Gelu,
                        bias=b1f[:, c * NE + n:c * NE + n + 1], scale=1.0,
                    )
                pending = (hact, targ, n, t0, t1)
        # ship experts 28..30 as soon as expert 30 flushes (during expert 31's
        # mm2), leaving only expert 31's 4 tiles for the final transfer
        nc.sync.dma_start(out=statsd[:, (NE - 4) * NT:(NE - 1) * NT, :],
                          in_=stats_sb[:, (NE - 4) * NT:(NE - 1) * NT, :])
        flush(pending)
        nc.sync.dma_start(out=statsd[:, (NE - 1) * NT:, :],
                          in_=stats_sb[:, (NE - 1) * NT:, :])
    return nc


LAST_RESULTS = None


def kernel(features, target_features, W1, b1, W2, b2):
    global LAST_RESULTS
    bf = ml_dtypes.bfloat16
    f8 = ml_dtypes.float8_e4m3
    features = np.asarray(features)
    target_features = np.asarray(target_features)
    W1 = np.asarray(W1)
    b1 = np.asarray(b1)
    W2 = np.asarray(W2)
    b2 = np.asarray(b2)

    # [C, NE, E, 2, BS] fp8: feat/targ interleaved per partition row
    feat4 = features.reshape(C, BS, NE, E).transpose(0, 2, 3, 1)
    targ4 = (target_features - b2[None]).reshape(C, BS, NE, E).transpose(0, 2, 3, 1)
    ft = np.stack([feat4, targ4], axis=3).astype(f8)   # [C, NE, E, 2, BS]
    w1h = W1.transpose(1, 0, 2).astype(bf)                          # [E, NE, H]
    w2h = W2.reshape(NE, 2, 128, E).transpose(2, 0, 1, 3).astype(f8)  # [128, NE, 2, E]
    b1h = np.ascontiguousarray(b1.reshape(NE, 2, 128).transpose(2, 1, 0).astype(np.float32))

    negi = (-np.eye(128)).astype(bf)
    head = np.ascontiguousarray(np.concatenate(
        [w1h[:, 0, :].view(np.uint16), negi.view(np.uint16),
         b1h.reshape(128, 64).view(np.uint16)],
        axis=1)).view(bf)

    nc = _build_nc()
    in_maps = [
        {"ft": np.ascontiguousarray(ft[c]),
         "w1": w1h, "w2": w2h, "head": head}
        for c in range(C)
    ]
    res = run_bass_kernel_spmd(nc, in_maps, list(range(C)))
    LAST_RESULTS = res
    # stats[p, tile] = [n0, mean0, M2_0, n1, mean1, M2_1] of the diff rows
    # (bn_stats splits the 512 free elems into two 256-halves);
    # sum of squares = M2_0 + n0*mean0^2 + M2_1 + n1*mean1^2.
    total = 0.0
    for r in res.results:
        st = r["stats"].astype(np.float64)
        total += (st[..., 2] + st[..., 0] * st[..., 1] ** 2
                  + st[..., 5] + st[..., 3] * st[..., 4] ** 2).sum()
    return np.array(total / (B * NE * E), dtype=np.float32)


# revision 27
# speedup vs baseline: 1.0069x; 1.0057x over previous
"""Trainium2 Bass kernel for nn_BaselineDistiller: grouped-expert MLP + MSE loss.

reference:
    h    = einsum('bne,neh->bnh', features, W1) + b1
    g    = gelu(h)                      # exact (erf) gelu
    pred = einsum('bnh,nhe->bne', g, W2) + b2
    out  = mean((pred - target)^2)

Strategy (8 NeuronCores, data-parallel over batch; ~151us on HW, vs 160us
for the bf16 predecessor):
  * Host: shard batch 8-ways; pre-transpose activations to expert-major
    [NE, E, 2, B_shard] fp8-e4m3 (feat and b2-folded target interleaved per
    partition row so one DMA per expert carries both, all on the SP hardware
    DMA queue). W1 stays bf16 (mm1 runs mixed bf16 lhsT x fp8 rhs -- verified
    exact on HW), W2 is fp8 packed [128, 2, E] for DoubleRow.
  * Device (per core, per expert, per pair of 512-col batch tiles):
      mm1 (2 H-chunks x 2 tiles)           -> h.T in PSUM (f32)
      ACT gelu(+b1 per-partition bias), one op per chunk over the pair
        (free dim 1024), output fp8 hact [128, tile, chunk, 512]
      mm2 accumulation group per tile: ONE DoubleRow fp8 matmul (K=2x128
        contracts both H-chunks in one pass, 2x PE throughput) + a
        (-I bf16) @ targ.T(fp8) matmul so PSUM holds pred.T - target.T
        (the subtract costs PE, not DVE)
      DVE bn_stats per diff tile -> per-partition {count, mean, M2} pairs
    The loop is software-pipelined (next pair's mm1 emitted before the
    current pair's mm2) so the in-order PE never idles waiting on gelu.
    A warmup block (DVE memset -> 8 dummy matmuls -> 1 dummy gelu) runs
    during the NRT preamble/first DMAs: it ramps the PE p-state and pulls
    the ~1.5us gelu ACT_TABLE_LOAD off the critical path, leaving the
    128-op ScalarE gelu stream (~132us span) fully gapless -- that stream
    is the saturated bottleneck engine (1 elem/lane/cycle at 1.2 GHz is a
    hard floor; FD=1024 per op is the largest PSUM double-buffering
    allows: 2 chunk regions + 2 pred banks = all 8 PSUM banks). PE
    (mm1 256 + mm2-DR 128 + negI 128 ops, ~128us) and DMA (~21 MB/core,
    one ~150 GB/s queue) run just under it. First expert splits feat/head/
    targ across the SP and ACT hardware queue families so mm1's inputs
    don't queue behind the full 512KB expert transfer. The remaining
    ~25us is NRT preamble (~7.6us), DMA/compute ramp, and the fixed NEFF
    exit barrier (~10us: the postamble clears ~250 semaphores, ~57
    EVENT_SEMAPHOREs per engine, regardless of kernel structure).
  * Host: sum of squares = sum over tiles of M2s + n*mean^2, reduced in f64,
    divided by the element count.

  Measured on this toolchain (bench_dr*.py):
    - matmul FD=512 costs ~218-262ns regardless of dtype/perf-mode (issue/
      ldweights-bound, 1 col/cycle); DoubleRow wins only by op-count (K=256
      per pass). Matmul out crossing a PSUM bank (FD>512 f32) is an ISA
      error. bn_stats FD is capped at 512 (hw limit).
    - ACTIVATE FD=1024 ~1110ns, FD=2048 ~1860ns (strided 2-region APs cost
      the same as contiguous) -- but 2x8KB gelu regions + pred banks exceed
      the 16KB PSUM, so FD=2048 schemes serialize the pipeline and lose.
    - gelu -> fp8 output is exact; mixed bf16 x fp8 matmul is exact.
    - walrus --enable-ldw-opt errors out on bass-emitted matmuls; shrinking
      DMAQueue num_queues stalls the rings (361us); DVE cannot issue DMAs.
"""

import contextlib
import ctypes
import json
import sys
import types

import ml_dtypes
import numpy as np

import concourse.bass as bass
import concourse.mybir as mybir
import concourse.tile as tile
from concourse import bass_utils
from concourse.bass import ts
from concourse.bass_utils import run_bass_kernel_spmd

B, NE, E, H = 16384, 32, 128, 256
C = 8              # cores
BS = B // C        # batch rows per core
BT = 512           # batch columns per matmul tile
NT = BS // BT
BF16 = mybir.dt.bfloat16
F32 = mybir.dt.float32
FP8 = mybir.dt.float8e4
DR = mybir.MatmulPerfMode.DoubleRow

# ---------------------------------------------------------------------------
# Environment shims (idempotent):
#  1. antenv.axon_hooks — the image's antenv lacks it; provide the NTFF
#     profile hook via ctypes so trace=True works when a caller requests it.
#  2. upload_artifacts — no bucket access in this container; keep local.
#  3. This walrus build rejects instructions with >1 sync-wait; split the
#     extra waits onto NoOps at BIR-serialization time.
# ---------------------------------------------------------------------------
_AXON_SO = "/opt/axon/libaxon_pjrt.so"


def _make_ntff_hook(so_path):
    try:
        lib = ctypes.CDLL(so_path)
    except OSError:
        return None
    if not hasattr(lib, "axon_start_nrt_profile"):
        return None
    lib.axon_start_nrt_profile.argtypes = [ctypes.POINTER(ctypes.c_int64), ctypes.c_size_t]
    lib.axon_start_nrt_profile.restype = ctypes.c_int64
    lib.axon_stop_nrt_profile.argtypes = [ctypes.c_char_p]
    lib.axon_stop_nrt_profile.restype = ctypes.c_int64

    @contextlib.contextmanager
    def _hook(output_dir, device_ids):
        import jax

        jax.devices()
        if device_ids:
            ids = (ctypes.c_int64 * len(device_ids))(*device_ids)
            rc = lib.axon_start_nrt_profile(ids, len(device_ids))
        else:
            rc = lib.axon_start_nrt_profile(None, 0)
        if rc != 0:
            raise RuntimeError(f"axon_start_nrt_profile rc={rc}")
        try:
            yield
        finally:
            n = lib.axon_stop_nrt_profile(str(output_dir).encode())
            print(f"profile: {n} file(s) written to {output_dir}", file=sys.stderr)

    return _hook


if "antenv.axon_hooks" not in sys.modules:
    _mod = types.ModuleType("antenv.axon_hooks")
    _the_hook = _make_ntff_hook(_AXON_SO)
    _mod.get_axon_ntff_profile_hook = lambda: _the_hook
    sys.modules["antenv.axon_hooks"] = _mod

bass_utils.upload_artifacts = lambda tmpdir: str(tmpdir)

_MAXW = 1
if not getattr(bass.Bass, "_wait_split_installed", False):
    _orig_to_json_bytes = bass.Bass.to_json_bytes

    def _split_sync_waits(self, *a, **kw):
        bir = json.loads(_orig_to_json_bytes(self, *a, **kw))
        for fn in bir.get("functions", []):
            for blk in fn.get("blocks", []):
                new_insts = []
                for inst in blk.get("instructions", []):
                    si = inst.get("sync_info") or {}
                    waits = si.get("on_wait") or []
                    if len(waits) > _MAXW:
                        extra, keep = waits[:-_MAXW], waits[-_MAXW:]
                        for k in range(0, len(extra), _MAXW):
                            new_insts.append({
                                "debug": inst.get("debug", 0),
                                "engine": inst["engine"],
                                "ins": [], "outs": [],
                                "name": f"{inst['name']}_wsplit{k}",
                                "opcode": "NoOp",
                                "sync_info": {"on_update": [],
                                              "on_wait": extra[k:k + _MAXW]},
                            })
                        si["on_wait"] = keep
                    new_insts.append(inst)
                blk["instructions"] = new_insts
        return json.dumps(bir).encode()

    bass.Bass.to_json_bytes = _split_sync_waits
    bass.Bass._wait_split_installed = True


# ---------------------------------------------------------------------------
# Device kernel
# ---------------------------------------------------------------------------
NTILES = NE * NT          # batch tiles, per core
STATS_DIM = 6


def _build_nc():
    nc = bass.Bass("TRN2", target_bir_lowering=False, debug=False)
    ftd = nc.declare_dram_parameter("ft", [NE, E, 2, BS], FP8, isOutput=False)
    w1d = nc.declare_dram_parameter("w1", [E, NE, H], BF16, isOutput=False)
    w2d = nc.declare_dram_parameter("w2", [128, NE, 2, E], FP8, isOutput=False)
    headd = nc.declare_dram_parameter("head", [128, 512], BF16, isOutput=False)
    statsd = nc.declare_dram_parameter("stats", [128, NTILES, STATS_DIM], F32,
                                       isOutput=True)

    with tile.TileContext(nc) as tc, contextlib.ExitStack() as ctx:
        wpool = ctx.enter_context(tc.tile_pool(name="weights", bufs=1))
        iopool = ctx.enter_context(tc.tile_pool(name="io", bufs=4))
        hpool = ctx.enter_context(tc.tile_pool(name="h", bufs=4))
        stpool = ctx.enter_context(tc.tile_pool(name="stats", bufs=1))
        ph0p = ctx.enter_context(tc.tile_pool(name="ph0", bufs=1, space="PSUM"))
        ph1p = ctx.enter_context(tc.tile_pool(name="ph1", bufs=1, space="PSUM"))
        ppp = ctx.enter_context(tc.tile_pool(name="pp", bufs=2, space="PSUM"))

        # Packed head tile = [expert-0 W1 | -I | b1-as-bits]: one DMA
        # unblocks the first matmuls and gelu bias. Expert-0 W2 rides the
        # first w2 group DMA (needed ~3us later, arrives ~2us in).
        head_sb = wpool.tile([128, 512], BF16)
        negi_sb = head_sb[:, 256:384]
        b1f = head_sb[:, 384:512].bitcast(F32)       # [128, 64] = b1[2, NE]
        GE = 4                      # experts per weight-DMA group
        NG = NE // GE
        GS = 4                      # experts per stats-out group
        w1g, w2g = [], []
        for g in range(NG):
            w1g.append(wpool.tile([E, GE, H], BF16, name=f"w1g{g}"))
            w2g.append(wpool.tile([128, GE, 2, E], FP8, name=f"w2g{g}"))

        stats_sb = stpool.tile([128, NTILES, STATS_DIM], F32)

        # Warmup while the first DMAs are in flight: 8 FD=512 matmuls hold
        # the PE busy ~3.5us so it reaches the full 2.4GHz p-state before
        # the first real mm1 (a shorter warmup leaves expert 0 at the mid
        # p-state and costs more than it saves), and a dummy gelu pulls the
        # ~1.5us ACT table load off the critical path. No data deps.
        warm = wpool.tile([128, 512], BF16, name="warm")
        nc.vector.memset(warm[:], 0.0)
        warm_pp = ppp.tile([128, BT], F32, name="pp0")
        for _ in range(8):
            nc.tensor.matmul(warm_pp[:], lhsT=warm[:, 0:128], rhs=warm[:],
                             start=True, stop=True)
        warm_out = wpool.tile([128, 512], BF16, name="warmo")
        nc.scalar.activation(warm_out[:], warm_pp[:],
                             mybir.ActivationFunctionType.Gelu, scale=1.0)

        # Software-pipelined over pairs of 512-col batch tiles: emit the NEXT
        # pair's mm1 before the CURRENT pair's mm2 so the PE (in-order) can
        # fill its gelu-wait with independent work.
        pending = None   # (hact, targ, n, t0, t1) awaiting mm2+bn_stats

        def flush(pending):
            hact, targ, n, t0, t1 = pending
            w2n = w2g[n // GE][:, n % GE]  # [128, 2, E]
            pp0 = ppp.tile([128, BT], F32, name="pp0")
            pp1 = ppp.tile([128, BT], F32, name="pp1")
            for pp_i, i in ((pp0, 0), (pp1, 1)):
                nc.tensor.matmul(pp_i[:], lhsT=w2n, rhs=hact[:, i],
                                 start=True, stop=False,
                                 perf_mode=DR, skip_group_check=True)
            for pp_i, t in ((pp0, t0), (pp1, t1)):
                nc.tensor.matmul(pp_i[:], lhsT=negi_sb,
                                 rhs=targ[:, ts(t, BT)],
                                 start=False, stop=True,
                                 skip_group_check=True)
            for pp_i, t in ((pp0, t0), (pp1, t1)):
                nc.vector.bn_stats(out=stats_sb[:, n * NT + t, :], in_=pp_i[:])

        for n in range(NE):
            if n == 0:
                # First expert: split feat/targ/head across the SP and ACT
                # hardware queue families so mm1's inputs (head + feat) don't
                # queue behind the full 512KB expert transfer.
                fa0 = iopool.tile([E, BS], FP8, name="fa0")
                ta0 = iopool.tile([E, BS], FP8, name="ta0")
                nc.scalar.dma_start(out=head_sb[:], in_=headd[:])
                nc.sync.dma_start(out=fa0[:], in_=ftd[0, :, 0, :])
                nc.scalar.dma_start(out=ta0[:], in_=ftd[0, :, 1, :])
                nc.sync.dma_start(out=w1g[0][:], in_=w1d[:, 0:GE, :])
                nc.sync.dma_start(out=w2g[0][:], in_=w2d[:, 0:GE, :, :])
                feat = fa0[:, :]
                featp = None
                targ = ta0[:, :]
            else:
                ft_sb = iopool.tile([E, 2, BS], FP8, tag="ft")
                nc.sync.dma_start(out=ft_sb[:], in_=ftd[n])
                feat = ft_sb[:, 0, :]
                featp = None
                targ = ft_sb[:, 1, :]
            if n % GE == 1:
                g = n // GE + 1
                if g < NG:
                    nc.sync.dma_start(out=w1g[g][:], in_=w1d[:, ts(g, GE), :])
            if n % GE == 2:
                g = n // GE + 1
                if g < NG:
                    nc.sync.dma_start(out=w2g[g][:], in_=w2d[:, ts(g, GE), :, :])
            if n % GS == 2 and n > GS:
                # experts <= n-2 have flushed; ship the previous group's stats
                gd = n // GS - 1
                nc.sync.dma_start(out=statsd[:, ts(gd, GS * NT), :],
                                  in_=stats_sb[:, ts(gd, GS * NT), :])
            if n == NE - 1:
                # experts 24..27 have flushed by now
                gd = NE // GS - 2
                nc.sync.dma_start(out=statsd[:, ts(gd, GS * NT), :],
                                  in_=stats_sb[:, ts(gd, GS * NT), :])
            for tp in range(NT // 2):
                t0, t1 = 2 * tp, 2 * tp + 1
                # mm1: h.T chunks; one weight load serves both tiles of a pair
                ph = [None, None]
                for c, pool_c in ((0, ph0p), (1, ph1p)):
                    ph[c] = pool_c.tile([128, 2, BT], F32, name=f"ph{c}")
                    for i, t in enumerate((t0, t1)):
                        nc.tensor.matmul(
                            ph[c][:, i, :],
                            lhsT=(head_sb[:, ts(c, 128)] if n == 0 else w1g[n // GE][:, n % GE, ts(c, 128)]),
                            rhs=(featp[t // 2][:, ts(t % 2, BT)]
                                 if featp is not None else feat[:, ts(t, BT)]),
                            start=True, stop=True,
                        )
                if pending is not None:
                    flush(pending)
                # gelu(+b1): one ACT op per chunk over both tiles (FD=1024),
                # fp8 output laid out [tile, chunk, BT] so each tile's rhs for
                # the DoubleRow mm2 is a [128, 2, BT] slice.
                hact = hpool.tile([128, 2, 2, BT], FP8)   # [tile, c, BT]
                for c in range(2):
                    nc.scalar.activation(
                        hact[:, :, c, :], ph[c][:, :, :],
                        mybir.ActivationFunctionType.Gelu,
                        bias=b1f[:, c * NE + n:c * NE + n + 1], scale=1.0,
                    )
                pending = (hact, targ, n, t0, t1)
        # ship experts 28..30 as soon as expert 30 flushes (during expert 31's
        # mm2), leaving only expert 31's 4 tiles for the final transfer
        nc.sync.dma_start(out=statsd[:, (NE - 4) * NT:(NE - 1) * NT, :],
                          in_=stats_sb[:, (NE - 4) * NT:(NE - 1) * NT, :])
        flush(pending)
        nc.sync.dma_start(out=statsd[:, (NE - 1) * NT:, :],
                          in_=stats_sb[:, (NE - 1) * NT:, :])
    return nc


LAST_RESULTS = None


def kernel(features, target_features, W1, b1, W2, b2):
    global LAST_RESULTS
    bf = ml_dtypes.bfloat16
    f8 = ml_dtypes.float8_e4m3
    features = np.asarray(features)
    target_features = np.asarray(target_features)
    W1 = np.asarray(W1)
    b1 = np.asarray(b1)
    W2 = np.asarray(W2)
    b2 = np.asarray(b2)

    # [C, NE, E, 2, BS] fp8: feat/targ interleaved per partition row
    feat4 = features.reshape(C, BS, NE, E).transpose(0, 2, 3, 1)
    targ4 = (target_features - b2[None]).reshape(C, BS, NE, E).transpose(0, 2, 3, 1)
    ft = np.stack([feat4, targ4], axis=3).astype(f8)   # [C, NE, E, 2, BS]
    w1h = W1.transpose(1, 0, 2).astype(bf)                          # [E, NE, H]
    w2h = W2.reshape(NE, 2, 128, E).transpose(2, 0, 1, 3).astype(f8)  # [128, NE, 2, E]
    b1h = np.ascontiguousarray(b1.reshape(NE, 2, 128).transpose(2, 1, 0).astype(np.float32))

    negi = (-np.eye(128)).astype(bf)
    head = np.ascontiguousarray(np.concatenate(
        [w1h[:, 0, :].view(np.uint16), negi.view(np.uint16),
         b1h.reshape(128, 64).view(np.uint16)],
        axis=1)).view(bf)

    nc = _build_nc()
    in_maps = [
        {"ft": np.ascontiguousarray(ft[c]),
         "w1": w1h, "w2": w2h, "head": head}
        for c in range(C)
    ]
    res = run_bass_kernel_spmd(nc, in_maps, list(range(C)))
    LAST_RESULTS = res
    # stats[p, tile] = [n0, mean0, M2_0, n1, mean1, M2_1] of the diff rows
    # (bn_stats splits the 512 free elems into two 256-halves);
    # sum of squares = M2_0 + n0*mean0^2 + M2_1 + n1*mean1^2.
    total = 0.0
    for r in res.results:
        st = r["stats"].astype(np.float64)
        total += (st[..., 2] + st[..., 0] * st[..., 1] ** 2
                  + st[..., 5] + st[..., 3] * st[..., 4] ** 2).sum()
    return np.array(total / (B * NE * E), dtype=np.float32)
